# revision 67
# baseline (speedup 1.0000x reference)
"""AnlaManifoldInpainter complex transformer on 8 trn2 cores, data-parallel over batch.

Layout: activations transposed [D on partitions (8x128), 512 tokens on free],
separate real/imag planes, residual kept in fp16 and consumed directly as
matmul rhs. Every complex projection (enc/dec/Q/K/V/WO/W1/W2) uses Karatsuba:
A = Wr.xr, B = Wi.xi, Cs = (Wr+Wi).(xr+xs); out_r = A-B, out_i = Cs-A-B.
RMSNorm is folded into the epilogues: matmuls consume the raw residual and the
per-token 1/rms (computed concurrently via ones-matmul partition reduction) is
multiplied in when draining PSUM, so the PE never waits on the norm chain.
Rotary + embedding gather happen host-side.
"""
import sys
sys.path.insert(0, "/opt/trn_rl_repo")

import numpy as np
from contextlib import ExitStack, nullcontext as _nullctx

import concourse.bass as bass
import concourse.tile as tile
from concourse import bacc, mybir
from concourse.bass_utils import run_bass_kernel_spmd

F32 = mybir.dt.float32
F16 = mybir.dt.float16
AF = mybir.ActivationFunctionType
ALU = mybir.AluOpType

V = 32000
D = 1024
H = 16
DH = 64
NB = 3
FF = 4 * D
B, S = 32, 128
EPS = 1e-6
NCORES = 8
BL = B // NCORES          # 4 sequences per core
T = BL * S                # 512 tokens per core
DT = D // 128             # 8 d-tiles
FT = FF // 128            # 32 f-tiles

_CACHE = {}
_LAST_EXEC_NS = None


def _prep_weights(inputs):
    """Host-side: rearrange weights into DMA-ready fp16 Karatsuba tile images."""
    w = {}

    def tiles_kxe(lhsT, name):
        # lhsT [K, E] -> [E/128 groups][128 p, K/128 kt, 128 c], contiguous per group
        K, E = lhsT.shape
        a = lhsT.reshape(K // 128, 128, E // 128, 128).transpose(2, 1, 0, 3)
        w[name] = np.ascontiguousarray(a, dtype=np.float16)

    def abc(Wc, name, gain=None):
        # complex unit, Karatsuba planes: Wr, Wi, Wr+Wi
        lhsT = Wc.T.copy()
        if gain is not None:
            lhsT = lhsT * gain[:, None]
        wr = lhsT.real.astype(np.float32)
        wi = lhsT.imag.astype(np.float32)
        tiles_kxe(wr, name + "r")
        tiles_kxe(wi, name + "i")
        tiles_kxe(wr + wi, name + "s")

    abc(np.asarray(inputs["enc_w"]), "enc")
    abc(np.asarray(inputs["dec_w"]), "dec")
    for i in range(NB):
        g1 = np.asarray(inputs["blk_norm1"][i], dtype=np.float32)
        g2 = np.asarray(inputs["blk_norm2"][i], dtype=np.float32)
        abc(np.asarray(inputs["blk_wq"][i]), f"q{i}_", g1)
        abc(np.asarray(inputs["blk_wk"][i]), f"k{i}_", g1)
        abc(np.asarray(inputs["blk_wv"][i]), f"v{i}_", g1)
        abc(np.asarray(inputs["blk_wo"][i]), f"o{i}_")
        abc(np.asarray(inputs["blk_w1"][i]), f"u{i}_", g2)
        abc(np.asarray(inputs["blk_w2"][i]), f"w{i}_")

    db = np.asarray(inputs["dec_b"])
    w["decbr"] = np.ascontiguousarray(db.real.reshape(DT, 128, 1), dtype=np.float32)
    w["decbi"] = np.ascontiguousarray(db.imag.reshape(DT, 128, 1), dtype=np.float32)
    eg = np.asarray(inputs["enc_g"], dtype=np.float32)
    w["encg"] = np.ascontiguousarray(eg.reshape(DT, 128, 1))
    w["ident"] = np.eye(128, dtype=np.float16)
    i2 = np.zeros((128, 64), dtype=np.float16)
    i2[0:64] = np.eye(64, dtype=np.float16)
    i2[64:128] = np.eye(64, dtype=np.float16)
    w["ident2"] = i2
    return w


def _build_nc(wshapes):
    nc = bacc.Bacc("TRN2", target_bir_lowering=False, debug=False, num_devices=NCORES)
    dram = {}
    for name, (shape, dt) in wshapes.items():
        dram[name] = nc.dram_tensor(name, list(shape), dt, kind="ExternalInput").ap()
    outr = nc.dram_tensor("outr", [DT, 128, T], F16, kind="ExternalOutput").ap()
    outi = nc.dram_tensor("outi", [DT, 128, T], F16, kind="ExternalOutput").ap()

    with tile.TileContext(nc) as tc:
        with ExitStack() as ctx:
            _body(ctx, tc, nc, dram, outr, outi)
    nc.compile()
    return nc


def _body(ctx, tc, nc, dram, outr, outi):
    zp = ctx.enter_context(tc.tile_pool(name="z", bufs=1))      # residual fp16 + sums
    ap_ = ctx.enter_context(tc.tile_pool(name="act", bufs=1))   # 96 activation slots
    wt = ctx.enter_context(tc.tile_pool(name="wt", bufs=1))     # weight stream
    tmp = ctx.enter_context(tc.tile_pool(name="tmp", bufs=2))   # temps
    sg = ctx.enter_context(tc.tile_pool(name="sg", bufs=1))     # singles
    ps = ctx.enter_context(tc.tile_pool(name="ps", bufs=2, space="PSUM"))

    # residual planes (fp16) + per-d sums for the Karatsuba s-plane
    zr = [zp.tile([128, T], F16, tag=f"zr{d}", name=f"zr{d}") for d in range(DT)]
    zi = [zp.tile([128, T], F16, tag=f"zi{d}", name=f"zi{d}") for d in range(DT)]
    zs = [zp.tile([128, T], F16, tag=f"zs{d}", name=f"zs{d}") for d in range(DT)]

    def slot(j, name):
        return ap_.tile([128, T], F16, tag=f"s{j}", name=name)

    ident = sg.tile([128, 128], F16, tag="ident", name="ident")
    ones16 = sg.tile([128, 1], F16, tag="ones", name="ones")
    ones11 = sg.tile([1, 1], F16, tag="ones11", name="ones11")
    nc.vector.memset(ones11, 1.0)
    nc.vector.memset(ones16, 1.0)
    ones1w = sg.tile([1, 128], F16, tag="ones1w", name="ones1w")
    nc.vector.memset(ones1w, 1.0)
    epsb = sg.tile([128, 1], F32, tag="epsb", name="epsb")
    nc.vector.memset(epsb, EPS)

    # ---------------- norm machinery (folded RMSNorm) ----------------
    class Norm:
        """Accumulate sum(|z|^2) over partition tiles via ones-matmul while the
        surrounding section computes; produce the per-token 1/rms broadcast."""

        def __init__(self, idx):
            self.idx = idx          # alternates 0/1 -> rinv tag
            self.ns = None
            self.nd = 0

        def issue_d(self, d, xr=None, xi=None):
            xr = zr[d] if xr is None else xr
            xi = zi[d] if xi is None else xi
            if self.ns is None:
                self.ns = ps.tile([1, T], F32, tag="O", name="nsum")
            for pl, xx in ((0, xr), (1, xi)):
                sq = tmp.tile([128, T], F16, tag="nsq", name="nsq", bufs=3)
                nc.vector.tensor_tensor(sq, xx, xx, op=ALU.mult)
                nc.tensor.matmul(self.ns, ones16, sq,
                                 start=(self.nd == 0 and pl == 0),
                                 stop=(self.nd == DT - 1 and pl == 1))
            self.nd += 1

        def finish(self, want16=True, want_t=False):
            assert self.nd == DT
            with tc.high_priority(offset=300):
                ssb = tmp.tile([1, T], F16, tag="ssb", name="ssb")
                nc.scalar.copy(ssb, self.ns)
                bc = ps.tile([128, T], F32, tag="O", name="nbc")
                nc.tensor.matmul(bc, ones1w, ssb, start=True, stop=True)
                rmst = tmp.tile([128, T], F32, tag="rmst", name="rmst")
                nc.scalar.activation(rmst, bc, AF.Sqrt, bias=epsb, scale=1.0 / D)
                rinv = tmp.tile([128, T], F32, tag=f"rinv{self.idx}",
                                name=f"rinv{self.idx}", bufs=1)
                nc.vector.reciprocal(rinv, rmst)
                rT = rT8 = None
                if want_t:
                    # per-token 1/rms with tokens on partitions: transpose the
                    # ssb row into a [128, BL] column block via 1-wide matmuls
                    rsT = ps.tile([128, BL], F32, tag="O", name="rsT")
                    for b in range(BL):
                        nc.tensor.matmul(rsT[:, b:b + 1], ssb[:, b * S:(b + 1) * S],
                                         ones11, start=True, stop=True)
                    rmsT = tmp.tile([128, BL], F32, tag="rmsT", name="rmsT")
                    nc.scalar.activation(rmsT, rsT, AF.Sqrt, bias=epsb, scale=1.0 / D)
                    rT = tmp.tile([128, BL], F32, tag=f"rT{self.idx}",
                                  name=f"rT{self.idx}", bufs=1)
                    nc.vector.reciprocal(rT, rmsT)
                    rT8 = tmp.tile([128, BL], F32, tag=f"rT8{self.idx}",
                                   name=f"rT8{self.idx}", bufs=1)
                    nc.vector.tensor_scalar(rT8, rT, 0.125, None, op0=ALU.mult)
                if not want16:
                    return rinv, None, rT, rT8
                rinv16 = tmp.tile([128, T], F16, tag=f"rh{self.idx}",
                                  name=f"rh{self.idx}", bufs=1)
                nc.vector.tensor_copy(rinv16, rinv)
            return rinv, rinv16, rT, rT8

    # ---------------- matmul + epilogue helpers ----------------
    def load3(prefix, g, nkt=8, hslice=None):
        out = []
        for pl in ("r", "i", "s"):
            t = wt.tile([128, nkt, 128], F16, tag=f"w{pl}", name=f"w{pl}", bufs=3)
            src = dram[prefix + pl][g]
            if hslice is not None:
                src = src[:, hslice]
            nc.sync.dma_start(t, src)
            out.append(t)
        return out

    def kara(wr, wi, ws, xr, xi, xs, nkt=8, k0=0, nktot=None, acc=None):
        """Issue 3*nkt matmuls; k0/nktot allow split-k accumulation. Pass a
        dict as `acc` to interleave two groups' accumulations."""
        nktot = nkt if nktot is None else nktot
        if k0 == 0:
            A = ps.tile([128, T], F32, tag="A", name="psA")
            Bp = ps.tile([128, T], F32, tag="B", name="psB")
            Cs = ps.tile([128, T], F32, tag="C", name="psC")
            if acc is None:
                kara.cur = (A, Bp, Cs)
            else:
                acc["cur"] = (A, Bp, Cs)
        A, Bp, Cs = kara.cur if acc is None else acc["cur"]
        for kt in range(nkt):
            k = k0 + kt
            nc.tensor.matmul(A, wr[:, kt], xr[k], start=(k == 0), stop=(k == nktot - 1))
        for kt in range(nkt):
            k = k0 + kt
            nc.tensor.matmul(Bp, wi[:, kt], xi[k], start=(k == 0), stop=(k == nktot - 1))
        for kt in range(nkt):
            k = k0 + kt
            nc.tensor.matmul(Cs, ws[:, kt], xs[k], start=(k == 0), stop=(k == nktot - 1))
        return A, Bp, Cs

    def epi_scale(A, Bp, Cs, rinvs, out_r, out_i):
        """out_r = (A-B)*rinv ; out_i = (Cs-A-B)*rinv. The A drain rides the
        Activation engine (Copy is in every table set); DVE only does the two
        unavoidable fp32-PSUM reads plus fast all-fp16 combines, so PSUM banks
        release promptly even when DVE queues twist work."""
        rinv, rinv16 = rinvs[0], rinvs[1]
        with tc.high_priority(offset=300):
            ca = tmp.tile([128, T], F16, tag="eta", name="eta")
            nc.scalar.copy(ca, A)
            tb = tmp.tile([128, T], F16, tag="etb", name="etb")
            nc.vector.tensor_tensor(tb, Bp, rinv, op=ALU.mult)
            tcs = tmp.tile([128, T], F16, tag="etc", name="etc")
            nc.vector.tensor_tensor(tcs, Cs, rinv, op=ALU.mult)
            tar = tmp.tile([128, T], F16, tag="etd", name="etd")
            nc.vector.tensor_tensor(tar, ca, rinv16, op=ALU.mult)
            nc.vector.tensor_tensor(out_r, tar, tb, op=ALU.subtract)
            s1 = tmp.tile([128, T], F16, tag="ete", name="ete")
            nc.vector.tensor_tensor(s1, tar, tb, op=ALU.add)
            nc.vector.tensor_tensor(out_i, tcs, s1, op=ALU.subtract)

    def epi_resid(A, Bp, Cs, zr_g, zi_g):
        """zr += A-B ; zi += Cs-A-B (residual update, no norm scale)."""
        with tc.high_priority(offset=300):
            ca = tmp.tile([128, T], F16, tag="eta", name="eta")
            nc.vector.tensor_copy(ca, A)
            tb = tmp.tile([128, T], F16, tag="etb", name="etb")
            nc.vector.tensor_copy(tb, Bp)
            m = tmp.tile([128, T], F16, tag="etd", name="etd")
            nc.vector.tensor_tensor(m, ca, tb, op=ALU.subtract)
            s1 = tmp.tile([128, T], F16, tag="ete", name="ete")
            nc.vector.tensor_tensor(s1, ca, tb, op=ALU.add)
            mi = tmp.tile([128, T], F16, tag="etc", name="etc")
            nc.vector.scalar_tensor_tensor(mi, Cs, 1.0, s1, op0=ALU.mult, op1=ALU.subtract)
            nc.vector.tensor_tensor(zr_g, m, zr_g, op=ALU.add)
            nc.vector.tensor_tensor(zi_g, mi, zi_g, op=ALU.add)

    def epi_plain_cat(A, Bp, Cs, cat_lo, cat_hi):
        """Like epi_plain but writes [re;im] head-cat tiles: A/B/Cs cover two
        heads (64 rows each); head j's re goes to cat_j[0:64], im to
        cat_j[64:128]."""
        with tc.high_priority(offset=300):
            ca = tmp.tile([128, T], F16, tag="eta", name="eta")
            nc.scalar.copy(ca, A)
            tb = tmp.tile([128, T], F16, tag="etb", name="etb")
            nc.vector.tensor_copy(tb, Bp)
            nc.vector.tensor_tensor(cat_lo[0:64, :], ca[0:64, :], tb[0:64, :],
                                    op=ALU.subtract)
            nc.vector.tensor_tensor(cat_hi[0:64, :], ca[64:128, :],
                                    tb[64:128, :], op=ALU.subtract)
            s1 = tmp.tile([128, T], F16, tag="ete", name="ete")
            nc.vector.tensor_tensor(s1, ca, tb, op=ALU.add)
            nc.vector.scalar_tensor_tensor(cat_lo[64:128, :], Cs[0:64, :], 1.0,
                                           s1[0:64, :], op0=ALU.mult,
                                           op1=ALU.subtract)
            nc.vector.scalar_tensor_tensor(cat_hi[64:128, :], Cs[64:128, :], 1.0,
                                           s1[64:128, :], op0=ALU.mult,
                                           op1=ALU.subtract)

    def epi_scale_cat(A, Bp, Cs, rinvs, cat_lo, cat_hi):
        rinv, rinv16 = rinvs[0], rinvs[1]
        with tc.high_priority(offset=300):
            ca = tmp.tile([128, T], F16, tag="eta", name="eta")
            nc.scalar.copy(ca, A)
            tb = tmp.tile([128, T], F16, tag="etb", name="etb")
            nc.vector.tensor_tensor(tb, Bp, rinv, op=ALU.mult)
            tcs = tmp.tile([128, T], F16, tag="etc", name="etc")
            nc.vector.tensor_tensor(tcs, Cs, rinv, op=ALU.mult)
            tar = tmp.tile([128, T], F16, tag="etd", name="etd")
            nc.vector.tensor_tensor(tar, ca, rinv16, op=ALU.mult)
            nc.vector.tensor_tensor(cat_lo[0:64, :], tar[0:64, :], tb[0:64, :],
                                    op=ALU.subtract)
            nc.vector.tensor_tensor(cat_hi[0:64, :], tar[64:128, :],
                                    tb[64:128, :], op=ALU.subtract)
            s1 = tmp.tile([128, T], F16, tag="ete", name="ete")
            nc.vector.tensor_tensor(s1, tar, tb, op=ALU.add)
            nc.vector.tensor_tensor(cat_lo[64:128, :], tcs[0:64, :],
                                    s1[0:64, :], op=ALU.subtract)
            nc.vector.tensor_tensor(cat_hi[64:128, :], tcs[64:128, :],
                                    s1[64:128, :], op=ALU.subtract)

    def epi_plain(A, Bp, Cs, out_r, out_i):
        """out_r = A-B ; out_i = Cs-A-B (no scale)."""
        with tc.high_priority(offset=300):
            ca = tmp.tile([128, T], F16, tag="eta", name="eta")
            nc.scalar.copy(ca, A)
            tb = tmp.tile([128, T], F16, tag="etb", name="etb")
            nc.vector.tensor_copy(tb, Bp)
            nc.vector.tensor_tensor(out_r, ca, tb, op=ALU.subtract)
            s1 = tmp.tile([128, T], F16, tag="ete", name="ete")
            nc.vector.tensor_tensor(s1, ca, tb, op=ALU.add)
            nc.vector.scalar_tensor_tensor(out_i, Cs, 1.0, s1, op0=ALU.mult,
                                           op1=ALU.subtract)

    # ---------------- encoder ----------------
    # z0 planes (host: gather + rotary) land in the first 24 activation slots.
    # First group's weights are queued ahead so the PE can start the moment
    # the z0r tiles land.
    enc_w0 = load3("enc", 0)
    z0r = [slot(d, f"z0r{d}") for d in range(DT)]
    z0i = [slot(8 + d, f"z0i{d}") for d in range(DT)]
    z0s = [slot(16 + d, f"z0s{d}") for d in range(DT)]
    for d in range(DT):
        nc.sync.dma_start(z0r[d], dram["z0r"][d])
    for d in range(DT):
        nc.sync.dma_start(z0i[d], dram["z0i"][d])
    for d in range(DT):
        nc.vector.tensor_tensor(z0s[d], z0r[d], z0i[d], op=ALU.add)
    nc.sync.dma_start(ident, dram["ident"])

    n_enc = Norm(0)
    for g in range(DT):
        wr, wi, ws = enc_w0 if g == 0 else load3("enc", g)
        A, Bp, Cs = kara(wr, wi, ws, z0r, z0i, z0s)
        # unscaled E written straight into the residual tiles
        with tc.high_priority(offset=60):
            ca = tmp.tile([128, T], F16, tag="eta", name="eta")
            nc.scalar.copy(ca, A)
            tb = tmp.tile([128, T], F16, tag="etb", name="etb")
            nc.vector.tensor_copy(tb, Bp)
            nc.vector.tensor_tensor(zr[g], ca, tb, op=ALU.subtract)
            s1 = tmp.tile([128, T], F16, tag="ete", name="ete")
            nc.vector.tensor_tensor(s1, ca, tb, op=ALU.add)
            nc.vector.scalar_tensor_tensor(zi[g], Cs, 1.0, s1, op0=ALU.mult, op1=ALU.subtract)
        n_enc.issue_d(g)
    rinv_e = n_enc.finish(want16=False)[0]
    # Block 0's folded norm consumes the RAW encoder output: the encoder's
    # per-token scale s cancels inside h = (E*s)/(s*rms(E)), so norm1 runs on
    # E directly and the scale materialization into the residual stream is
    # deferred — the tile WAR deps push it after Q/K/V's reads and before
    # WO's residual update, entirely off the PE critical path.
    n1 = Norm(1)
    for d in range(DT):
        nc.vector.tensor_tensor(zs[d], zr[d], zi[d], op=ALU.add)
        n1.issue_d(d)
    rinv1 = n1.finish(want_t=True)

    def enc_scale_now():
        # materialize the encoder norm into the residual stream (issued
        # after block-0 Q/K/V so those consume raw E with matching rinv)
        for d in range(DT):
            gt = sg.tile([128, 1], F32, tag=f"encg{d}", name=f"encg{d}")
            nc.sync.dma_start(gt, dram["encg"][d])
            comb = tmp.tile([128, T], F32, tag="rmst", name="comb")
            nc.vector.tensor_scalar(comb, rinv_e, gt, None, op0=ALU.mult)
            nc.vector.tensor_tensor(zr[d], zr[d], comb, op=ALU.mult)
            nc.vector.tensor_tensor(zi[d], zi[d], comb, op=ALU.mult)

    # ---------------- transformer blocks ----------------
    for i in range(NB):
        qcat = [slot(ht, f"qcat{ht}") for ht in range(H)]
        kcat = [slot(16 + ht, f"kcat{ht}") for ht in range(H)]
        vr = [slot(32 + g, f"vr{g}") for g in range(DT)]
        vi = [slot(40 + g, f"vi{g}") for g in range(DT)]
        or_ = [slot(48 + g, f"or{g}") for g in range(DT)]
        oi_ = [slot(56 + g, f"oi{g}") for g in range(DT)]
        os_ = [slot(64 + g, f"os{g}") for g in range(DT)]

        # --- Q/K/V projections (Karatsuba). Q stays unscaled (1/rms rides
        # the Exp scale pointer); K is scaled by rinv1; V is computed
        # TRANSPOSED ([token, dim], z-slices as stationary) so attention needs
        # no V transposes, and its 1/rms rides the at-drain. ---
        rT1, rT81 = rinv1[2], rinv1[3]
        for g in range(DT):
            wr, wi, ws = load3(f"q{i}_", g)
            A, Bp, Cs = kara(wr, wi, ws, zr, zi, zs)
            epi_plain_cat(A, Bp, Cs, qcat[2 * g], qcat[2 * g + 1])
            wr, wi, ws = load3(f"k{i}_", g)
            A, Bp, Cs = kara(wr, wi, ws, zr, zi, zs)
            epi_scale_cat(A, Bp, Cs, rinv1, kcat[2 * g], kcat[2 * g + 1])
            wr, wi, ws = load3(f"v{i}_", g)
            A = ps.tile([128, T], F32, tag="A", name="psA")
            Bp = ps.tile([128, T], F32, tag="B", name="psB")
            Cs = ps.tile([128, T], F32, tag="C", name="psC")
            for b in range(BL):
                sl = slice(b * S, (b + 1) * S)
                for kt in range(DT):
                    nc.tensor.matmul(A[:, sl], zr[kt][:, sl], wr[:, kt],
                                     start=(kt == 0), stop=(kt == DT - 1))
                for kt in range(DT):
                    nc.tensor.matmul(Bp[:, sl], zi[kt][:, sl], wi[:, kt],
                                     start=(kt == 0), stop=(kt == DT - 1))
                for kt in range(DT):
                    nc.tensor.matmul(Cs[:, sl], zs[kt][:, sl], ws[:, kt],
                                     start=(kt == 0), stop=(kt == DT - 1))
            epi_plain(A, Bp, Cs, vr[g], vi[g])

        # --- attention units: one head at a time, all 4 seqs batched into
        # [128, T] psum tiles (8x pipeline depth vs per-seq tiles) ---
        for ht in range(H):
          with tc.high_priority(offset=320):
            g2, half = ht // 2, (ht % 2) * 64
            lg4 = ps.tile([128, T], F32, tag="A" if ht % 2 == 0 else "B",
                          name="psLG")
            for b in range(BL):
                sl = slice(b * S, (b + 1) * S)
                nc.tensor.matmul(lg4[:, sl], qcat[ht][:, sl], kcat[ht][:, sl],
                                 start=True, stop=True)
            aexp = tmp.tile([128, T], F16, tag="aexp", name="aexp")
            dens = []
            for b in range(BL):
                sl = slice(b * S, (b + 1) * S)
                den = tmp.tile([128, 1], F32, tag="den", name="den", bufs=4)
                nc.scalar.activation(aexp[:, sl], lg4[:, sl], AF.Exp,
                                     scale=rT81[:, b:b + 1], accum_out=den)
                dens.append(den)
            anrm = tmp.tile([128, T], F16, tag="anrm", name="anrm")
            if ht % 2 == 0:
                at2_ps = ps.tile([128, 2 * T], F16, tag="C", name="psAT2")
                _body.at2 = at2_ps
            at_ps = _body.at2[:, (ht % 2) * T:(ht % 2 + 1) * T]
            for b in range(BL):
                sl = slice(b * S, (b + 1) * S)
                rec = tmp.tile([128, 1], F32, tag="rec", name="rec", bufs=4)
                with tc.high_priority(offset=650):
                    nc.vector.reciprocal(rec, dens[b])
                    nc.vector.tensor_scalar(anrm[:, sl], aexp[:, sl], rec, None,
                                            op0=ALU.mult)
                nc.tensor.transpose(at_ps[:, sl], anrm[:, sl], ident)
            at = tmp.tile([128, T], F16, tag="at", name="at")
            for b in range(BL):
                sl = slice(b * S, (b + 1) * S)
                with tc.high_priority(offset=650):
                    nc.vector.tensor_scalar(at[:, sl], at_ps[:, sl],
                                            rT1[:, b:b + 1], None, op0=ALU.mult)
            o_ps = ps.tile([128, T], F32, tag="O", name="psO")
            for b in range(BL):
                sl = slice(b * S, (b + 1) * S)
                c0 = b * S + half
                nc.tensor.matmul(o_ps[0:64, sl], vr[g2][:, c0:c0 + 64],
                                 at[:, sl], start=True, stop=True)
                nc.tensor.matmul(o_ps[64:128, sl], vi[g2][:, c0:c0 + 64],
                                 at[:, sl], start=True, stop=True)
            nc.scalar.copy(or_[g2][half:half + 64, :], o_ps[0:64, :])
            nc.vector.tensor_copy(oi_[g2][half:half + 64, :], o_ps[64:128, :])
            if ht % 2 == 1:
                nc.gpsimd.tensor_tensor(os_[g2], or_[g2], oi_[g2], op=ALU.add)

        # --- WO + residual; norm2 interleaved ---
        if i == 0:
            enc_scale_now()
        n2 = Norm(0)
        for g in range(DT):
            wr, wi, ws = load3(f"o{i}_", g)
            A, Bp, Cs = kara(wr, wi, ws, or_, oi_, os_)
            epi_resid(A, Bp, Cs, zr[g], zi[g])
            nc.vector.tensor_tensor(zs[g], zr[g], zi[g], op=ALU.add)
            n2.issue_d(g)
        rinv2 = n2.finish()

        # --- FF1 + PhaseTwist ---
        h1r = [slot(fg, f"h1r{fg}") for fg in range(FT)]
        h1i = [slot(32 + fg, f"h1i{fg}") for fg in range(FT)]
        h1s = [slot(64 + fg, f"h1s{fg}") for fg in range(FT)]
        for fg0 in range(0, FT, 4):
            staged = []
            for j in range(4):
                fg = fg0 + j
                wr, wi, ws = load3(f"u{i}_", fg)
                A, Bp, Cs = kara(wr, wi, ws, zr, zi, zs)
                epi_scale(A, Bp, Cs, rinv2, h1r[fg], h1i[fg])
                sq = tmp.tile([128, T], F16, tag="twsq", name="twsq", bufs=4)
                nc.vector.tensor_tensor(sq, h1r[fg], h1r[fg], op=ALU.mult)
                sqi = tmp.tile([128, T], F16, tag="tsqi", name="tsqi")
                nc.vector.tensor_tensor(sqi, h1i[fg], h1i[fg], op=ALU.mult)
                nc.vector.tensor_tensor(sq, sq, sqi, op=ALU.add)
                staged.append((fg, sq))
            rs = []
            for fg, sq in staged:
                r = tmp.tile([128, T], F16, tag="twr", name="twr", bufs=4)
                nc.scalar.activation(r, sq, AF.Sqrt)
                rs.append(r)
            uvs = []
            twist_pri = 150 if fg0 == FT - 4 else 0
            for (fg, sq), r in zip(staged, rs):
                sn = tmp.tile([128, T], F16, tag="twu", name="twu", bufs=4)
                nc.scalar.activation(sn, r, AF.Sin)
                v = tmp.tile([128, T], F16, tag="twv", name="twv", bufs=4)
                nc.scalar.activation(v, r, AF.Sin, scale=0.5)
                uvs.append((sn, v))
            eng_m = nc.vector if fg0 == FT - 4 else nc.gpsimd
            for (fg, sq), (sn, v) in zip(staged, uvs):
              with tc.high_priority(offset=twist_pri) if twist_pri else _nullctx():
                cs = tmp.tile([128, T], F16, tag="twcs", name="twcs")
                nc.vector.tensor_tensor(cs, v, v, op=ALU.mult)
                nc.vector.tensor_scalar(cs, cs, -2.0, 1.0, op0=ALU.mult, op1=ALU.add)
                m1 = tmp.tile([128, T], F16, tag="m1", name="m1")
                eng_m.tensor_tensor(m1, h1r[fg], cs, op=ALU.mult)
                m3 = tmp.tile([128, T], F16, tag="m3", name="m3")
                eng_m.tensor_tensor(m3, h1r[fg], sn, op=ALU.mult)
                m2 = tmp.tile([128, T], F16, tag="m2", name="m2")
                nc.vector.tensor_tensor(m2, h1i[fg], sn, op=ALU.mult)
                m4 = tmp.tile([128, T], F16, tag="m4", name="m4")
                nc.vector.tensor_tensor(m4, h1i[fg], cs, op=ALU.mult)
                nc.vector.tensor_tensor(h1r[fg], m1, m2, op=ALU.subtract)
                nc.vector.tensor_tensor(h1i[fg], m3, m4, op=ALU.add)
                eng_m.tensor_tensor(h1s[fg], h1r[fg], h1i[fg], op=ALU.add)

        # --- W2 + residual; next norm1 interleaved ---
        n1 = Norm(1) if i < NB - 1 else None
        for g0 in range(0, DT, 2):
            accs = ({}, {})
            for q in range(4):
                for j in (0, 1):
                    g = g0 + j
                    wr, wi, ws = load3(f"w{i}_", g,
                                       hslice=slice(q * 8, (q + 1) * 8))
                    kara(wr, wi, ws, h1r, h1i, h1s,
                         k0=q * 8, nktot=FT, acc=accs[j])
            for j in (0, 1):
                g = g0 + j
                A, Bp, Cs = accs[j]["cur"]
                epi_resid(A, Bp, Cs, zr[g], zi[g])
                nc.vector.tensor_tensor(zs[g], zr[g], zi[g], op=ALU.add)
                if n1 is not None:
                    n1.issue_d(g)
        if n1 is not None:
            rinv1 = n1.finish(want_t=True)

    # ---------------- decoder ----------------
    for g in range(DT):
        wr, wi, ws = load3("dec", g)
        br = sg.tile([128, 1], F32, tag=f"dbr{g}", name=f"dbr{g}")
        bi = sg.tile([128, 1], F32, tag=f"dbi{g}", name=f"dbi{g}")
        nc.sync.dma_start(br, dram["decbr"][g])
        nc.sync.dma_start(bi, dram["decbi"][g])
        A, Bp, Cs = kara(wr, wi, ws, zr, zi, zs)
        with tc.high_priority(offset=60):
            ca = tmp.tile([128, T], F16, tag="eta", name="eta")
            nc.scalar.copy(ca, A)
            tb = tmp.tile([128, T], F16, tag="etb", name="etb")
            nc.vector.tensor_copy(tb, Bp)
            m = tmp.tile([128, T], F16, tag="etd", name="etd")
            nc.vector.tensor_tensor(m, ca, tb, op=ALU.subtract)
            s1 = tmp.tile([128, T], F16, tag="ete", name="ete")
            nc.vector.tensor_tensor(s1, ca, tb, op=ALU.add)
            mi = tmp.tile([128, T], F16, tag="etc", name="etc")
            nc.vector.scalar_tensor_tensor(mi, Cs, 1.0, s1, op0=ALU.mult, op1=ALU.subtract)
        ot = tmp.tile([128, T], F16, tag="eta", name="dor")
        nc.vector.tensor_scalar(ot, m, br, None, op0=ALU.add)
        nc.sync.dma_start(outr[g], ot)
        oti = tmp.tile([128, T], F16, tag="etb", name="doi")
        nc.vector.tensor_scalar(oti, mi, bi, None, op0=ALU.add)
        nc.sync.dma_start(outi[g], oti)


def _host_z0(inputs):
    """Embedding gather + complex rotary on the host; returns fp16 planes."""
    emb = np.asarray(inputs["emb"])
    x = np.asarray(inputs["x"])
    z0 = emb[x]                                      # (B, S, D) complex64
    pos = np.arange(S, dtype=np.float64)
    inv_freq = np.exp(-np.arange(D, dtype=np.float64) / D * np.log(10000.0))
    ang = pos[:, None] * inv_freq[None, :]           # [S, D]
    rot = (np.cos(ang) + 1j * np.sin(ang))
    z0 = (z0 * rot[None, :, :]).astype(np.complex64)
    return z0


def kernel(**inputs):
    w = _prep_weights(inputs)
    z0 = _host_z0(inputs)

    wshapes = {k: (v.shape, F16 if v.dtype == np.float16 else F32) for k, v in w.items()}
    for nm in ("z0r", "z0i"):
        wshapes[nm] = ((DT, 128, T), F16)

    if "nc" not in _CACHE:
        _CACHE["nc"] = _build_nc(wshapes)
    nc = _CACHE["nc"]

    core_maps = []
    for c in range(NCORES):
        zc = z0[c * BL:(c + 1) * BL]                 # (4, 128, 1024)
        zt = zc.reshape(T, D).T                      # (1024, 512) d-major
        m = dict(w)
        m["z0r"] = np.ascontiguousarray(zt.real, dtype=np.float16).reshape(DT, 128, T)
        m["z0i"] = np.ascontiguousarray(zt.imag, dtype=np.float16).reshape(DT, 128, T)
        core_maps.append(m)

    import os
    trace = bool(os.environ.get("KTRACE"))
    res = run_bass_kernel_spmd(nc, core_maps, core_ids=list(range(NCORES)),
                               trace=trace)
    global _LAST_EXEC_NS
    _LAST_EXEC_NS = res.exec_time_ns
    out = np.empty((B, S, D), dtype=np.complex64)
    for c in range(NCORES):
        orr = res.results[c]["outr"].reshape(D, T).astype(np.float32)
        oii = res.results[c]["outi"].reshape(D, T).astype(np.float32)
        oc = (orr + 1j * oii).astype(np.complex64)   # [D, T]
        out[c * BL:(c + 1) * BL] = oc.T.reshape(BL, S, D)
    return out


# revision 70
# speedup vs baseline: 1.0022x; 1.0022x over previous
"""AnlaManifoldInpainter complex transformer on 8 trn2 cores, data-parallel over batch.

Layout: activations transposed [D on partitions (8x128), 512 tokens on free],
separate real/imag planes, residual kept in fp16 and consumed directly as
matmul rhs. Every complex projection (enc/dec/Q/K/V/WO/W1/W2) uses Karatsuba:
A = Wr.xr, B = Wi.xi, Cs = (Wr+Wi).(xr+xs); out_r = A-B, out_i = Cs-A-B.
RMSNorm is folded into the epilogues: matmuls consume the raw residual and the
per-token 1/rms (computed concurrently via ones-matmul partition reduction) is
multiplied in when draining PSUM, so the PE never waits on the norm chain.
Rotary + embedding gather happen host-side.
"""
import sys
sys.path.insert(0, "/opt/trn_rl_repo")

import numpy as np
from contextlib import ExitStack, nullcontext as _nullctx

import concourse.bass as bass
import concourse.tile as tile
from concourse import bacc, mybir
from concourse.bass_utils import run_bass_kernel_spmd

F32 = mybir.dt.float32
F16 = mybir.dt.float16
AF = mybir.ActivationFunctionType
ALU = mybir.AluOpType

V = 32000
D = 1024
H = 16
DH = 64
NB = 3
FF = 4 * D
B, S = 32, 128
EPS = 1e-6
NCORES = 8
BL = B // NCORES          # 4 sequences per core
T = BL * S                # 512 tokens per core
DT = D // 128             # 8 d-tiles
FT = FF // 128            # 32 f-tiles

_CACHE = {}
_LAST_EXEC_NS = None


def _prep_weights(inputs):
    """Host-side: rearrange weights into DMA-ready fp16 Karatsuba tile images."""
    w = {}

    def tiles_kxe(lhsT, name):
        # lhsT [K, E] -> [E/128 groups][128 p, K/128 kt, 128 c], contiguous per group
        K, E = lhsT.shape
        a = lhsT.reshape(K // 128, 128, E // 128, 128).transpose(2, 1, 0, 3)
        w[name] = np.ascontiguousarray(a, dtype=np.float16)

    def abc(Wc, name, gain=None):
        # complex unit, Karatsuba planes: Wr, Wi, Wr+Wi
        lhsT = Wc.T.copy()
        if gain is not None:
            lhsT = lhsT * gain[:, None]
        wr = lhsT.real.astype(np.float32)
        wi = lhsT.imag.astype(np.float32)
        tiles_kxe(wr, name + "r")
        tiles_kxe(wi, name + "i")
        tiles_kxe(wr + wi, name + "s")

    abc(np.asarray(inputs["enc_w"]), "enc")
    abc(np.asarray(inputs["dec_w"]), "dec")
    for i in range(NB):
        g1 = np.asarray(inputs["blk_norm1"][i], dtype=np.float32)
        g2 = np.asarray(inputs["blk_norm2"][i], dtype=np.float32)
        abc(np.asarray(inputs["blk_wq"][i]), f"q{i}_", g1)
        abc(np.asarray(inputs["blk_wk"][i]), f"k{i}_", g1)
        abc(np.asarray(inputs["blk_wv"][i]), f"v{i}_", g1)
        abc(np.asarray(inputs["blk_wo"][i]), f"o{i}_")
        abc(np.asarray(inputs["blk_w1"][i]), f"u{i}_", g2)
        abc(np.asarray(inputs["blk_w2"][i]), f"w{i}_")

    db = np.asarray(inputs["dec_b"])
    w["decbr"] = np.ascontiguousarray(db.real.reshape(DT, 128, 1), dtype=np.float32)
    w["decbi"] = np.ascontiguousarray(db.imag.reshape(DT, 128, 1), dtype=np.float32)
    eg = np.asarray(inputs["enc_g"], dtype=np.float32)
    w["encg"] = np.ascontiguousarray(eg.reshape(DT, 128, 1))
    w["ident"] = np.eye(128, dtype=np.float16)
    i2 = np.zeros((128, 64), dtype=np.float16)
    i2[0:64] = np.eye(64, dtype=np.float16)
    i2[64:128] = np.eye(64, dtype=np.float16)
    w["ident2"] = i2
    return w


def _build_nc(wshapes):
    nc = bacc.Bacc("TRN2", target_bir_lowering=False, debug=False, num_devices=NCORES)
    dram = {}
    for name, (shape, dt) in wshapes.items():
        dram[name] = nc.dram_tensor(name, list(shape), dt, kind="ExternalInput").ap()
    outr = nc.dram_tensor("outr", [DT, 128, T], F16, kind="ExternalOutput").ap()
    outi = nc.dram_tensor("outi", [DT, 128, T], F16, kind="ExternalOutput").ap()

    with tile.TileContext(nc) as tc:
        with ExitStack() as ctx:
            _body(ctx, tc, nc, dram, outr, outi)
    nc.compile()
    return nc


def _body(ctx, tc, nc, dram, outr, outi):
    zp = ctx.enter_context(tc.tile_pool(name="z", bufs=1))      # residual fp16 + sums
    ap_ = ctx.enter_context(tc.tile_pool(name="act", bufs=1))   # 96 activation slots
    wt = ctx.enter_context(tc.tile_pool(name="wt", bufs=1))     # weight stream
    tmp = ctx.enter_context(tc.tile_pool(name="tmp", bufs=2))   # temps
    sg = ctx.enter_context(tc.tile_pool(name="sg", bufs=1))     # singles
    ps = ctx.enter_context(tc.tile_pool(name="ps", bufs=2, space="PSUM"))

    # residual planes (fp16) + per-d sums for the Karatsuba s-plane
    zr = [zp.tile([128, T], F16, tag=f"zr{d}", name=f"zr{d}") for d in range(DT)]
    zi = [zp.tile([128, T], F16, tag=f"zi{d}", name=f"zi{d}") for d in range(DT)]
    zs = [zp.tile([128, T], F16, tag=f"zs{d}", name=f"zs{d}") for d in range(DT)]

    def slot(j, name):
        return ap_.tile([128, T], F16, tag=f"s{j}", name=name)

    ident = sg.tile([128, 128], F16, tag="ident", name="ident")
    ones16 = sg.tile([128, 1], F16, tag="ones", name="ones")
    ones11 = sg.tile([1, 1], F16, tag="ones11", name="ones11")
    nc.vector.memset(ones11, 1.0)
    nc.vector.memset(ones16, 1.0)
    ones1w = sg.tile([1, 128], F16, tag="ones1w", name="ones1w")
    nc.vector.memset(ones1w, 1.0)
    epsb = sg.tile([128, 1], F32, tag="epsb", name="epsb")
    nc.vector.memset(epsb, EPS)

    # ---------------- norm machinery (folded RMSNorm) ----------------
    class Norm:
        """Accumulate sum(|z|^2) over partition tiles via ones-matmul while the
        surrounding section computes; produce the per-token 1/rms broadcast."""

        def __init__(self, idx):
            self.idx = idx          # alternates 0/1 -> rinv tag
            self.ns = None
            self.nd = 0

        def issue_d(self, d, xr=None, xi=None):
            xr = zr[d] if xr is None else xr
            xi = zi[d] if xi is None else xi
            if self.ns is None:
                self.ns = ps.tile([1, T], F32, tag="O", name="nsum")
            for pl, xx in ((0, xr), (1, xi)):
                sq = tmp.tile([128, T], F16, tag="nsq", name="nsq", bufs=3)
                nc.vector.tensor_tensor(sq, xx, xx, op=ALU.mult)
                nc.tensor.matmul(self.ns, ones16, sq,
                                 start=(self.nd == 0 and pl == 0),
                                 stop=(self.nd == DT - 1 and pl == 1))
            self.nd += 1

        def finish(self, want16=True, want_t=False):
            assert self.nd == DT
            with tc.high_priority(offset=300):
                ssb = tmp.tile([1, T], F16, tag="ssb", name="ssb")
                nc.scalar.copy(ssb, self.ns)
                bc = ps.tile([128, T], F32, tag="O", name="nbc")
                nc.tensor.matmul(bc, ones1w, ssb, start=True, stop=True)
                rmst = tmp.tile([128, T], F32, tag="rmst", name="rmst")
                nc.scalar.activation(rmst, bc, AF.Sqrt, bias=epsb, scale=1.0 / D)
                rinv = tmp.tile([128, T], F32, tag=f"rinv{self.idx}",
                                name=f"rinv{self.idx}", bufs=1)
                nc.vector.reciprocal(rinv, rmst)
                rT = rT8 = None
                if want_t:
                    # per-token 1/rms with tokens on partitions: transpose the
                    # ssb row into a [128, BL] column block via 1-wide matmuls
                    rsT = ps.tile([128, BL], F32, tag="O", name="rsT")
                    for b in range(BL):
                        nc.tensor.matmul(rsT[:, b:b + 1], ssb[:, b * S:(b + 1) * S],
                                         ones11, start=True, stop=True)
                    rmsT = tmp.tile([128, BL], F32, tag="rmsT", name="rmsT")
                    nc.scalar.activation(rmsT, rsT, AF.Sqrt, bias=epsb, scale=1.0 / D)
                    rT = tmp.tile([128, BL], F32, tag=f"rT{self.idx}",
                                  name=f"rT{self.idx}", bufs=1)
                    nc.vector.reciprocal(rT, rmsT)
                    rT8 = tmp.tile([128, BL], F32, tag=f"rT8{self.idx}",
                                   name=f"rT8{self.idx}", bufs=1)
                    nc.vector.tensor_scalar(rT8, rT, 0.125, None, op0=ALU.mult)
                if not want16:
                    return rinv, None, rT, rT8
                rinv16 = tmp.tile([128, T], F16, tag=f"rh{self.idx}",
                                  name=f"rh{self.idx}", bufs=1)
                nc.vector.tensor_copy(rinv16, rinv)
            return rinv, rinv16, rT, rT8

    # ---------------- matmul + epilogue helpers ----------------
    def load3(prefix, g, nkt=8, hslice=None):
        out = []
        for pl in ("r", "i", "s"):
            t = wt.tile([128, nkt, 128], F16, tag=f"w{pl}", name=f"w{pl}", bufs=3)
            src = dram[prefix + pl][g]
            if hslice is not None:
                src = src[:, hslice]
            nc.sync.dma_start(t, src)
            out.append(t)
        return out

    def kara(wr, wi, ws, xr, xi, xs, nkt=8, k0=0, nktot=None, acc=None):
        """Issue 3*nkt matmuls; k0/nktot allow split-k accumulation. Pass a
        dict as `acc` to interleave two groups' accumulations."""
        nktot = nkt if nktot is None else nktot
        if k0 == 0:
            A = ps.tile([128, T], F32, tag="A", name="psA")
            Bp = ps.tile([128, T], F32, tag="B", name="psB")
            Cs = ps.tile([128, T], F32, tag="C", name="psC")
            if acc is None:
                kara.cur = (A, Bp, Cs)
            else:
                acc["cur"] = (A, Bp, Cs)
        A, Bp, Cs = kara.cur if acc is None else acc["cur"]
        for kt in range(nkt):
            k = k0 + kt
            nc.tensor.matmul(A, wr[:, kt], xr[k], start=(k == 0), stop=(k == nktot - 1))
        for kt in range(nkt):
            k = k0 + kt
            nc.tensor.matmul(Bp, wi[:, kt], xi[k], start=(k == 0), stop=(k == nktot - 1))
        for kt in range(nkt):
            k = k0 + kt
            nc.tensor.matmul(Cs, ws[:, kt], xs[k], start=(k == 0), stop=(k == nktot - 1))
        return A, Bp, Cs

    def epi_scale(A, Bp, Cs, rinvs, out_r, out_i):
        """out_r = (A-B)*rinv ; out_i = (Cs-A-B)*rinv. The A drain rides the
        Activation engine (Copy is in every table set); DVE only does the two
        unavoidable fp32-PSUM reads plus fast all-fp16 combines, so PSUM banks
        release promptly even when DVE queues twist work."""
        rinv, rinv16 = rinvs[0], rinvs[1]
        with tc.high_priority(offset=300):
            ca = tmp.tile([128, T], F16, tag="eta", name="eta")
            nc.scalar.copy(ca, A)
            tb = tmp.tile([128, T], F16, tag="etb", name="etb")
            nc.vector.tensor_tensor(tb, Bp, rinv, op=ALU.mult)
            tcs = tmp.tile([128, T], F16, tag="etc", name="etc")
            nc.vector.tensor_tensor(tcs, Cs, rinv, op=ALU.mult)
            tar = tmp.tile([128, T], F16, tag="etd", name="etd")
            nc.vector.tensor_tensor(tar, ca, rinv16, op=ALU.mult)
            nc.vector.tensor_tensor(out_r, tar, tb, op=ALU.subtract)
            s1 = tmp.tile([128, T], F16, tag="ete", name="ete")
            nc.vector.tensor_tensor(s1, tar, tb, op=ALU.add)
            nc.vector.tensor_tensor(out_i, tcs, s1, op=ALU.subtract)

    def epi_resid(A, Bp, Cs, zr_g, zi_g):
        """zr += A-B ; zi += Cs-A-B (residual update, no norm scale)."""
        with tc.high_priority(offset=300):
            ca = tmp.tile([128, T], F16, tag="eta", name="eta")
            nc.vector.tensor_copy(ca, A)
            tb = tmp.tile([128, T], F16, tag="etb", name="etb")
            nc.vector.tensor_copy(tb, Bp)
            m = tmp.tile([128, T], F16, tag="etd", name="etd")
            nc.vector.tensor_tensor(m, ca, tb, op=ALU.subtract)
            s1 = tmp.tile([128, T], F16, tag="ete", name="ete")
            nc.vector.tensor_tensor(s1, ca, tb, op=ALU.add)
            mi = tmp.tile([128, T], F16, tag="etc", name="etc")
            nc.vector.scalar_tensor_tensor(mi, Cs, 1.0, s1, op0=ALU.mult, op1=ALU.subtract)
            nc.vector.tensor_tensor(zr_g, m, zr_g, op=ALU.add)
            nc.vector.tensor_tensor(zi_g, mi, zi_g, op=ALU.add)

    def epi_plain_cat(A, Bp, Cs, cat_lo, cat_hi):
        """Like epi_plain but writes [re;im] head-cat tiles: A/B/Cs cover two
        heads (64 rows each); head j's re goes to cat_j[0:64], im to
        cat_j[64:128]."""
        with tc.high_priority(offset=300):
            ca = tmp.tile([128, T], F16, tag="eta", name="eta")
            nc.scalar.copy(ca, A)
            tb = tmp.tile([128, T], F16, tag="etb", name="etb")
            nc.vector.tensor_copy(tb, Bp)
            nc.vector.tensor_tensor(cat_lo[0:64, :], ca[0:64, :], tb[0:64, :],
                                    op=ALU.subtract)
            nc.vector.tensor_tensor(cat_hi[0:64, :], ca[64:128, :],
                                    tb[64:128, :], op=ALU.subtract)
            s1 = tmp.tile([128, T], F16, tag="ete", name="ete")
            nc.vector.tensor_tensor(s1, ca, tb, op=ALU.add)
            nc.vector.scalar_tensor_tensor(cat_lo[64:128, :], Cs[0:64, :], 1.0,
                                           s1[0:64, :], op0=ALU.mult,
                                           op1=ALU.subtract)
            nc.vector.scalar_tensor_tensor(cat_hi[64:128, :], Cs[64:128, :], 1.0,
                                           s1[64:128, :], op0=ALU.mult,
                                           op1=ALU.subtract)

    def epi_scale_cat(A, Bp, Cs, rinvs, cat_lo, cat_hi):
        rinv, rinv16 = rinvs[0], rinvs[1]
        with tc.high_priority(offset=300):
            ca = tmp.tile([128, T], F16, tag="eta", name="eta")
            nc.scalar.copy(ca, A)
            tb = tmp.tile([128, T], F16, tag="etb", name="etb")
            nc.vector.tensor_tensor(tb, Bp, rinv, op=ALU.mult)
            tcs = tmp.tile([128, T], F16, tag="etc", name="etc")
            nc.vector.tensor_tensor(tcs, Cs, rinv, op=ALU.mult)
            tar = tmp.tile([128, T], F16, tag="etd", name="etd")
            nc.vector.tensor_tensor(tar, ca, rinv16, op=ALU.mult)
            nc.vector.tensor_tensor(cat_lo[0:64, :], tar[0:64, :], tb[0:64, :],
                                    op=ALU.subtract)
            nc.vector.tensor_tensor(cat_hi[0:64, :], tar[64:128, :],
                                    tb[64:128, :], op=ALU.subtract)
            s1 = tmp.tile([128, T], F16, tag="ete", name="ete")
            nc.vector.tensor_tensor(s1, tar, tb, op=ALU.add)
            nc.vector.tensor_tensor(cat_lo[64:128, :], tcs[0:64, :],
                                    s1[0:64, :], op=ALU.subtract)
            nc.vector.tensor_tensor(cat_hi[64:128, :], tcs[64:128, :],
                                    s1[64:128, :], op=ALU.subtract)

    def epi_plain(A, Bp, Cs, out_r, out_i):
        """out_r = A-B ; out_i = Cs-A-B (no scale)."""
        with tc.high_priority(offset=300):
            ca = tmp.tile([128, T], F16, tag="eta", name="eta")
            nc.scalar.copy(ca, A)
            tb = tmp.tile([128, T], F16, tag="etb", name="etb")
            nc.vector.tensor_copy(tb, Bp)
            nc.vector.tensor_tensor(out_r, ca, tb, op=ALU.subtract)
            s1 = tmp.tile([128, T], F16, tag="ete", name="ete")
            nc.vector.tensor_tensor(s1, ca, tb, op=ALU.add)
            nc.vector.scalar_tensor_tensor(out_i, Cs, 1.0, s1, op0=ALU.mult,
                                           op1=ALU.subtract)

    # ---------------- encoder ----------------
    # z0 planes (host: gather + rotary) land in the first 24 activation slots.
    # First group's weights are queued ahead so the PE can start the moment
    # the z0r tiles land.
    enc_w0 = load3("enc", 0)
    z0r = [slot(d, f"z0r{d}") for d in range(DT)]
    z0i = [slot(8 + d, f"z0i{d}") for d in range(DT)]
    z0s = [slot(16 + d, f"z0s{d}") for d in range(DT)]
    for d in range(DT):
        nc.sync.dma_start(z0r[d], dram["z0r"][d])
    for d in range(DT):
        nc.sync.dma_start(z0i[d], dram["z0i"][d])
    for d in range(DT):
        nc.vector.tensor_tensor(z0s[d], z0r[d], z0i[d], op=ALU.add)
    nc.sync.dma_start(ident, dram["ident"])

    n_enc = Norm(0)
    for g in range(DT):
        wr, wi, ws = enc_w0 if g == 0 else load3("enc", g)
        A, Bp, Cs = kara(wr, wi, ws, z0r, z0i, z0s)
        # unscaled E written straight into the residual tiles
        with tc.high_priority(offset=60):
            ca = tmp.tile([128, T], F16, tag="eta", name="eta")
            nc.scalar.copy(ca, A)
            tb = tmp.tile([128, T], F16, tag="etb", name="etb")
            nc.vector.tensor_copy(tb, Bp)
            nc.vector.tensor_tensor(zr[g], ca, tb, op=ALU.subtract)
            s1 = tmp.tile([128, T], F16, tag="ete", name="ete")
            nc.vector.tensor_tensor(s1, ca, tb, op=ALU.add)
            nc.vector.scalar_tensor_tensor(zi[g], Cs, 1.0, s1, op0=ALU.mult, op1=ALU.subtract)
        n_enc.issue_d(g)
    rinv_e = n_enc.finish(want16=False)[0]
    # Block 0's folded norm consumes the RAW encoder output: the encoder's
    # per-token scale s cancels inside h = (E*s)/(s*rms(E)), so norm1 runs on
    # E directly and the scale materialization into the residual stream is
    # deferred — the tile WAR deps push it after Q/K/V's reads and before
    # WO's residual update, entirely off the PE critical path.
    n1 = Norm(1)
    for d in range(DT):
        nc.vector.tensor_tensor(zs[d], zr[d], zi[d], op=ALU.add)
        n1.issue_d(d)
    rinv1 = n1.finish(want_t=True)

    def enc_scale_now():
        # materialize the encoder norm into the residual stream (issued
        # after block-0 Q/K/V so those consume raw E with matching rinv)
        for d in range(DT):
            gt = sg.tile([128, 1], F32, tag=f"encg{d}", name=f"encg{d}")
            nc.sync.dma_start(gt, dram["encg"][d])
            comb = tmp.tile([128, T], F32, tag="rmst", name="comb")
            nc.vector.tensor_scalar(comb, rinv_e, gt, None, op0=ALU.mult)
            nc.vector.tensor_tensor(zr[d], zr[d], comb, op=ALU.mult)
            nc.vector.tensor_tensor(zi[d], zi[d], comb, op=ALU.mult)

    # ---------------- transformer blocks ----------------
    for i in range(NB):
        qcat = [slot(ht, f"qcat{ht}") for ht in range(H)]
        kcat = [slot(16 + ht, f"kcat{ht}") for ht in range(H)]
        vr = [slot(32 + g, f"vr{g}") for g in range(DT)]
        vi = [slot(40 + g, f"vi{g}") for g in range(DT)]
        or_ = [slot(48 + g, f"or{g}") for g in range(DT)]
        oi_ = [slot(56 + g, f"oi{g}") for g in range(DT)]
        os_ = [slot(64 + g, f"os{g}") for g in range(DT)]

        # --- Q/K/V projections (Karatsuba). Q stays unscaled (1/rms rides
        # the Exp scale pointer); K is scaled by rinv1; V is computed
        # TRANSPOSED ([token, dim], z-slices as stationary) so attention needs
        # no V transposes, and its 1/rms rides the at-drain. ---
        rT1, rT81 = rinv1[2], rinv1[3]
        for g in range(DT):
            wr, wi, ws = load3(f"q{i}_", g)
            A, Bp, Cs = kara(wr, wi, ws, zr, zi, zs)
            epi_plain_cat(A, Bp, Cs, qcat[2 * g], qcat[2 * g + 1])
            wr, wi, ws = load3(f"k{i}_", g)
            A, Bp, Cs = kara(wr, wi, ws, zr, zi, zs)
            epi_scale_cat(A, Bp, Cs, rinv1, kcat[2 * g], kcat[2 * g + 1])
            wr, wi, ws = load3(f"v{i}_", g)
            A = ps.tile([128, T], F32, tag="A", name="psA")
            Bp = ps.tile([128, T], F32, tag="B", name="psB")
            Cs = ps.tile([128, T], F32, tag="C", name="psC")
            for b in range(BL):
                sl = slice(b * S, (b + 1) * S)
                for kt in range(DT):
                    nc.tensor.matmul(A[:, sl], zr[kt][:, sl], wr[:, kt],
                                     start=(kt == 0), stop=(kt == DT - 1))
                for kt in range(DT):
                    nc.tensor.matmul(Bp[:, sl], zi[kt][:, sl], wi[:, kt],
                                     start=(kt == 0), stop=(kt == DT - 1))
                for kt in range(DT):
                    nc.tensor.matmul(Cs[:, sl], zs[kt][:, sl], ws[:, kt],
                                     start=(kt == 0), stop=(kt == DT - 1))
            epi_plain(A, Bp, Cs, vr[g], vi[g])

        # --- attention units: one head at a time, all 4 seqs batched into
        # [128, T] psum tiles (8x pipeline depth vs per-seq tiles) ---
        for ht in range(H):
          with tc.high_priority(offset=320):
            g2, half = ht // 2, (ht % 2) * 64
            lg4 = ps.tile([128, T], F32, tag="A" if ht % 2 == 0 else "B",
                          name="psLG")
            for b in range(BL):
                sl = slice(b * S, (b + 1) * S)
                nc.tensor.matmul(lg4[:, sl], qcat[ht][:, sl], kcat[ht][:, sl],
                                 start=True, stop=True)
            aexp = tmp.tile([128, T], F16, tag="aexp", name="aexp")
            dens = []
            for b in range(BL):
                sl = slice(b * S, (b + 1) * S)
                den = tmp.tile([128, 1], F32, tag="den", name="den", bufs=4)
                nc.scalar.activation(aexp[:, sl], lg4[:, sl], AF.Exp,
                                     scale=rT81[:, b:b + 1], accum_out=den)
                dens.append(den)
            anrm = tmp.tile([128, T], F16, tag="anrm", name="anrm")
            if ht % 2 == 0:
                at2_ps = ps.tile([128, 2 * T], F16, tag="C", name="psAT2")
                _body.at2 = at2_ps
            at_ps = _body.at2[:, (ht % 2) * T:(ht % 2 + 1) * T]
            for b in range(BL):
                sl = slice(b * S, (b + 1) * S)
                rec = tmp.tile([128, 1], F32, tag="rec", name="rec", bufs=4)
                with tc.high_priority(offset=650):
                    nc.vector.reciprocal(rec, dens[b])
                    # fused: aexp * rec (softmax denom) * rinv_k (V-side norm)
                    nc.vector.scalar_tensor_tensor(anrm[:, sl], aexp[:, sl],
                                                   rec, rinv1[1][:, sl],
                                                   op0=ALU.mult, op1=ALU.mult)
                nc.tensor.transpose(at_ps[:, sl], anrm[:, sl], ident)
            at = tmp.tile([128, T], F16, tag="at", name="at")
            with tc.high_priority(offset=650):
                nc.vector.tensor_copy(at, at_ps)
            o_ps = ps.tile([128, T], F32, tag="O", name="psO")
            for b in range(BL):
                sl = slice(b * S, (b + 1) * S)
                c0 = b * S + half
                nc.tensor.matmul(o_ps[0:64, sl], vr[g2][:, c0:c0 + 64],
                                 at[:, sl], start=True, stop=True)
                nc.tensor.matmul(o_ps[64:128, sl], vi[g2][:, c0:c0 + 64],
                                 at[:, sl], start=True, stop=True)
            nc.scalar.copy(or_[g2][half:half + 64, :], o_ps[0:64, :])
            nc.vector.tensor_copy(oi_[g2][half:half + 64, :], o_ps[64:128, :])
            if ht % 2 == 1:
                nc.gpsimd.tensor_tensor(os_[g2], or_[g2], oi_[g2], op=ALU.add)

        # --- WO + residual; norm2 interleaved ---
        if i == 0:
            enc_scale_now()
        n2 = Norm(0)
        for g in range(DT):
            wr, wi, ws = load3(f"o{i}_", g)
            A, Bp, Cs = kara(wr, wi, ws, or_, oi_, os_)
            epi_resid(A, Bp, Cs, zr[g], zi[g])
            nc.vector.tensor_tensor(zs[g], zr[g], zi[g], op=ALU.add)
            n2.issue_d(g)
        rinv2 = n2.finish()

        # --- FF1 + PhaseTwist ---
        h1r = [slot(fg, f"h1r{fg}") for fg in range(FT)]
        h1i = [slot(32 + fg, f"h1i{fg}") for fg in range(FT)]
        h1s = [slot(64 + fg, f"h1s{fg}") for fg in range(FT)]
        for fg0 in range(0, FT, 4):
            staged = []
            for j in range(4):
                fg = fg0 + j
                wr, wi, ws = load3(f"u{i}_", fg)
                A, Bp, Cs = kara(wr, wi, ws, zr, zi, zs)
                epi_scale(A, Bp, Cs, rinv2, h1r[fg], h1i[fg])
                sq = tmp.tile([128, T], F16, tag="twsq", name="twsq", bufs=4)
                nc.vector.tensor_tensor(sq, h1r[fg], h1r[fg], op=ALU.mult)
                sqi = tmp.tile([128, T], F16, tag="tsqi", name="tsqi")
                nc.vector.tensor_tensor(sqi, h1i[fg], h1i[fg], op=ALU.mult)
                nc.vector.tensor_tensor(sq, sq, sqi, op=ALU.add)
                staged.append((fg, sq))
            rs = []
            for fg, sq in staged:
                r = tmp.tile([128, T], F16, tag="twr", name="twr", bufs=4)
                nc.scalar.activation(r, sq, AF.Sqrt)
                rs.append(r)
            uvs = []
            twist_pri = 150 if fg0 == FT - 4 else 0
            for (fg, sq), r in zip(staged, rs):
                sn = tmp.tile([128, T], F16, tag="twu", name="twu", bufs=4)
                nc.scalar.activation(sn, r, AF.Sin)
                v = tmp.tile([128, T], F16, tag="twv", name="twv", bufs=4)
                nc.scalar.activation(v, r, AF.Sin, scale=0.5)
                uvs.append((sn, v))
            eng_m = nc.vector if fg0 == FT - 4 else nc.gpsimd
            for (fg, sq), (sn, v) in zip(staged, uvs):
              with tc.high_priority(offset=twist_pri) if twist_pri else _nullctx():
                cs = tmp.tile([128, T], F16, tag="twcs", name="twcs")
                nc.vector.tensor_tensor(cs, v, v, op=ALU.mult)
                nc.vector.tensor_scalar(cs, cs, -2.0, 1.0, op0=ALU.mult, op1=ALU.add)
                m1 = tmp.tile([128, T], F16, tag="m1", name="m1")
                eng_m.tensor_tensor(m1, h1r[fg], cs, op=ALU.mult)
                m3 = tmp.tile([128, T], F16, tag="m3", name="m3")
                eng_m.tensor_tensor(m3, h1r[fg], sn, op=ALU.mult)
                m2 = tmp.tile([128, T], F16, tag="m2", name="m2")
                nc.vector.tensor_tensor(m2, h1i[fg], sn, op=ALU.mult)
                m4 = tmp.tile([128, T], F16, tag="m4", name="m4")
                nc.vector.tensor_tensor(m4, h1i[fg], cs, op=ALU.mult)
                nc.vector.tensor_tensor(h1r[fg], m1, m2, op=ALU.subtract)
                nc.vector.tensor_tensor(h1i[fg], m3, m4, op=ALU.add)
                eng_m.tensor_tensor(h1s[fg], h1r[fg], h1i[fg], op=ALU.add)

        # --- W2 + residual; next norm1 interleaved ---
        n1 = Norm(1) if i < NB - 1 else None
        for g0 in range(0, DT, 2):
            accs = ({}, {})
            for q in range(4):
                for j in (0, 1):
                    g = g0 + j
                    wr, wi, ws = load3(f"w{i}_", g,
                                       hslice=slice(q * 8, (q + 1) * 8))
                    kara(wr, wi, ws, h1r, h1i, h1s,
                         k0=q * 8, nktot=FT, acc=accs[j])
            for j in (0, 1):
                g = g0 + j
                A, Bp, Cs = accs[j]["cur"]
                epi_resid(A, Bp, Cs, zr[g], zi[g])
                nc.vector.tensor_tensor(zs[g], zr[g], zi[g], op=ALU.add)
                if n1 is not None:
                    n1.issue_d(g)
        if n1 is not None:
            rinv1 = n1.finish(want_t=True)

    # ---------------- decoder ----------------
    for g in range(DT):
        wr, wi, ws = load3("dec", g)
        br = sg.tile([128, 1], F32, tag=f"dbr{g}", name=f"dbr{g}")
        bi = sg.tile([128, 1], F32, tag=f"dbi{g}", name=f"dbi{g}")
        nc.sync.dma_start(br, dram["decbr"][g])
        nc.sync.dma_start(bi, dram["decbi"][g])
        A, Bp, Cs = kara(wr, wi, ws, zr, zi, zs)
        with tc.high_priority(offset=60):
            ca = tmp.tile([128, T], F16, tag="eta", name="eta")
            nc.scalar.copy(ca, A)
            tb = tmp.tile([128, T], F16, tag="etb", name="etb")
            nc.vector.tensor_copy(tb, Bp)
            m = tmp.tile([128, T], F16, tag="etd", name="etd")
            nc.vector.tensor_tensor(m, ca, tb, op=ALU.subtract)
            s1 = tmp.tile([128, T], F16, tag="ete", name="ete")
            nc.vector.tensor_tensor(s1, ca, tb, op=ALU.add)
            mi = tmp.tile([128, T], F16, tag="etc", name="etc")
            nc.vector.scalar_tensor_tensor(mi, Cs, 1.0, s1, op0=ALU.mult, op1=ALU.subtract)
        ot = tmp.tile([128, T], F16, tag="eta", name="dor")
        nc.vector.tensor_scalar(ot, m, br, None, op0=ALU.add)
        nc.sync.dma_start(outr[g], ot)
        oti = tmp.tile([128, T], F16, tag="etb", name="doi")
        nc.vector.tensor_scalar(oti, mi, bi, None, op0=ALU.add)
        nc.sync.dma_start(outi[g], oti)


def _host_z0(inputs):
    """Embedding gather + complex rotary on the host; returns fp16 planes."""
    emb = np.asarray(inputs["emb"])
    x = np.asarray(inputs["x"])
    z0 = emb[x]                                      # (B, S, D) complex64
    pos = np.arange(S, dtype=np.float64)
    inv_freq = np.exp(-np.arange(D, dtype=np.float64) / D * np.log(10000.0))
    ang = pos[:, None] * inv_freq[None, :]           # [S, D]
    rot = (np.cos(ang) + 1j * np.sin(ang))
    z0 = (z0 * rot[None, :, :]).astype(np.complex64)
    return z0


def kernel(**inputs):
    w = _prep_weights(inputs)
    z0 = _host_z0(inputs)

    wshapes = {k: (v.shape, F16 if v.dtype == np.float16 else F32) for k, v in w.items()}
    for nm in ("z0r", "z0i"):
        wshapes[nm] = ((DT, 128, T), F16)

    if "nc" not in _CACHE:
        _CACHE["nc"] = _build_nc(wshapes)
    nc = _CACHE["nc"]

    core_maps = []
    for c in range(NCORES):
        zc = z0[c * BL:(c + 1) * BL]                 # (4, 128, 1024)
        zt = zc.reshape(T, D).T                      # (1024, 512) d-major
        m = dict(w)
        m["z0r"] = np.ascontiguousarray(zt.real, dtype=np.float16).reshape(DT, 128, T)
        m["z0i"] = np.ascontiguousarray(zt.imag, dtype=np.float16).reshape(DT, 128, T)
        core_maps.append(m)

    import os
    trace = bool(os.environ.get("KTRACE"))
    res = run_bass_kernel_spmd(nc, core_maps, core_ids=list(range(NCORES)),
                               trace=trace)
    global _LAST_EXEC_NS
    _LAST_EXEC_NS = res.exec_time_ns
    out = np.empty((B, S, D), dtype=np.complex64)
    for c in range(NCORES):
        orr = res.results[c]["outr"].reshape(D, T).astype(np.float32)
        oii = res.results[c]["outi"].reshape(D, T).astype(np.float32)
        oc = (orr + 1j * oii).astype(np.complex64)   # [D, T]
        out[c * BL:(c + 1) * BL] = oc.T.reshape(BL, S, D)
    return out
